# revision 1
# baseline (speedup 1.0000x reference)
"""Trainium2 Bass kernel for the MACE-style symmetric contraction:

    out  = einsum("xyik,kc,bci->bcxy", U3, w3, nf)
    c2   = einsum("xyk,kc->cxy", U2, w2)[None] + out
    out  = einsum("bcxi,bci->bcx", c2, nf)
    c1   = einsum("xk,kc->cx", U1, w1)[None] + out
    out  = einsum("bci,bci->bc", c1, nf)

Algebraically:

    out[b,c] =   sum_{x,y,i} W3U[x,y,i,c] nf[b,c,x] nf[b,c,y] nf[b,c,i]
               + sum_{x,y}   U2w2[c,x,y]  nf[b,c,x] nf[b,c,y]
               + sum_{x}     U1w1[c,x]    nf[b,c,x]

with W3U = einsum("xyik,kc->xyic", U3, w3).  U2/U1 fold into the triple
product via an augmented i row (i'=48 holds U2w2; (i'=48,y'=48) holds
U1w1) and a constant-1 channel.

Sharding: irrep axis x (48) split 6-per-core across 8 NeuronCores (this
splits the dominant HBM stream, U3, 8 ways).  Host sums the 8 partial
[512, 96] outputs.

Device pipeline (all fp16 except PSUM/scan state, which are fp32):
  build:   W3U[c, (i,x,y')] = w3.T @ u3t on PE, k-accumulated in PSUM,
           drained fp16 to a DRAM scratch.
  phase B: per c-pair + b-tile: Z[b,(x,y')] = nfa.T @ W3U_c on PE; then
           a fused multiply+prefix-sum (custom DVE MAC-scan, or
           ACT-drain + Pool multiply + Pool scan) gives running sums of
           Z*nfy whose row boundaries are the per-x group sums.  The
           per-x difference of boundaries is folded into the host-side
           dnfx = nf[x] - nf[x+1] (Abel summation), so
           out[b,c] = sum_x bnd[b,c,x] * dnfx[b,c,x].
"""

import numpy as np

B = 512          # atoms
C = 96           # feats
I = 48           # irreps
K3, K2, K1 = 1270, 24, 3
NCORES = 8
XS = I // NCORES  # 6 x-values per core
Y1 = I + 1        # 49: y plus augmentation column
I1 = I + 1        # 49: i plus augmentation row
KP = 1280         # K3 padded to 10 partition tiles
NX = XS * Y1      # 294
MP = I * XS * Y1  # 14112  (m = (h, i, xl, y'))
XSH = XS // 2     # 3: x-values per half
NXH = XSH * Y1    # 147
MPH = I * XSH * Y1  # 7056: build columns per half
MCHUNK = 1024
NMC = (MP + MCHUNK - 1) // MCHUNK  # 14 (last chunk 800)
KT = KP // 128                     # 10
PAIRS = C // 2                     # 48
NT = B // 128                      # 4 b-tiles

_CACHE = {}

# Per-c-pair phase-B path: 'D' = DVE MAC-scan, 'Q' = ACT+Pool+DVE-reduce.
# Must match between _build_nc (engine choice) and _prep_inputs (final
# weights: dnfx for 'D', nfx for 'Q').
_PATTERN = ['D' for cp in range(PAIRS)]

# exec time of the last device run (ns), when BASS_TRACE=1
LAST_EXEC_NS = None


def _register_mac_scan():
    """Custom DVE op: out[t] = prefix-sum of in0[t]*in1[t] (fp32 state).
    Fuses phase B's elementwise multiply and its y'-group reduction into
    one DVE pass; group sums are recovered from the running sum at row
    boundaries."""
    import concourse.dve_ops as dve_ops_mod
    if any(op.name == "TT_MAC_SCAN_ANT" for op in dve_ops_mod.OPS):
        return next(op for op in dve_ops_mod.OPS
                    if op.name == "TT_MAC_SCAN_ANT")
    from concourse.dve_spec import Spec, scan, Src0, Src1
    from concourse.dve_uop import AluOp
    from concourse.dve_ops import DveOp

    def _ref_mac_scan(in0, in1, s0, s1, imm2):
        p = in0.astype(np.float32) * in1.astype(np.float32)
        return np.cumsum(p.reshape(p.shape[0], -1), axis=1).reshape(
            p.shape).astype(np.float32)

    spec = Spec(body=scan(AluOp.ADD, Src0 * Src1), reference=_ref_mac_scan)
    op = DveOp("TT_MAC_SCAN_ANT", spec, subdim=False,
               uops_sha={"v3": "b3fc3e78a862b7eb",
                         "v4": "bc6a002865d48b97"})
    dve_ops_mod.OPS.append(op)
    dve_ops_mod.CUSTOM_DVE_SPECS[op.name] = spec
    dve_ops_mod._SUB_OPCODE_FOR_NAME[op.name] = (
        max(dve_ops_mod._SUB_OPCODE_FOR_NAME.values()) + 1)
    return op


INTERLEAVE = False


def _build_nc(debug=None):
    import concourse.bass as bass
    import concourse.mybir as mybir
    from concourse.tile import TileContext

    mac_scan = _register_mac_scan()

    f16 = mybir.dt.float16
    f32 = mybir.dt.float32
    mult = mybir.AluOpType.mult
    add = mybir.AluOpType.add
    bypass = mybir.AluOpType.bypass

    import concourse.bacc as bacc
    nc = bacc.Bacc(None, target_bir_lowering=False)
    u3t = nc.dram_tensor("u3t", [KP, MP], f16, kind="ExternalInput")
    w3p = nc.dram_tensor("w3p", [KP, C], f16, kind="ExternalInput")
    nfa = nc.dram_tensor("nfa", [128, PAIRS * B], f16, kind="ExternalInput")
    nfy = nc.dram_tensor("nfy", [B, C * I1], f16, kind="ExternalInput")
    # final-pass weights: dnfx (Abel) for 'D' c-pairs, plain nfx for 'Q'
    # c-pairs -- host-built to match PATTERN
    fwx = nc.dram_tensor("fwx", [B, C * XS], f16, kind="ExternalInput")
    # nfc1[b, h*PAIRS+cp] = nf[b, 2cp+1, xlo+3h] -- cancels the per-half
    # scan's c0->c1 carry
    nfc1 = nc.dram_tensor("nfc1", [B, 2 * PAIRS], f16, kind="ExternalInput")
    u2aug = nc.dram_tensor("u2aug", [32, NX], f16, kind="ExternalInput")
    w21 = nc.dram_tensor("w21", [32, C], f16, kind="ExternalInput")
    outp = nc.dram_tensor("out", [B, C], f32, kind="ExternalOutput")

    with TileContext(nc) as tc:
        with (
            nc.allow_low_precision(reason="fp16 intermediates; rel-err "
                                   "budget 2e-2 vs ~1e-3 incurred"),
            tc.tile_pool(name="dram", bufs=1, space="DRAM") as dpool,
            tc.tile_pool(name="const", bufs=1) as cpool,
            tc.tile_pool(name="u3", bufs=12) as u3pool,
            tc.tile_pool(name="psum", bufs=2, space="PSUM") as psum,
            tc.tile_pool(name="zpsum", bufs=2, space="PSUM") as zpsum,
            tc.tile_pool(name="lt", bufs=6) as ltpool,
            tc.tile_pool(name="sc", bufs=6) as scpool,
            tc.tile_pool(name="rep", bufs=6) as reppool,
            tc.tile_pool(name="stg", bufs=3) as stgpool,
            tc.tile_pool(name="fin", bufs=2) as finpool,
        ):
            # scratch row c = [(i'=0..47) from the U3 build | (i'=48) aug]
            w3u_scr = dpool.tile([C, I1 * NX], f16)

            # ---- resident constants (w3 first; bulk loads are emitted
            # after the first build chunk so they don't delay u3 tiles) ----
            w3sb = cpool.tile([128, KT * C], f16)
            w3v = w3sb[:, :].rearrange("p (k c) -> p k c", c=C)
            nc.sync.dma_start(
                out=w3v[:, :, :],
                in_=w3p[:, :].rearrange("(k p) c -> p k c", p=128))
            w21sb = cpool.tile([32, C], f16)
            nc.sync.dma_start(out=w21sb[:, :], in_=w21[:, :])
            u2sb = cpool.tile([32, NX], f16)
            nc.sync.dma_start(out=u2sb[:, :], in_=u2aug[:, :])
            nfasb = cpool.tile([128, PAIRS * B], f16)
            nfav = nfasb[:, :].rearrange("p (cp b) -> p cp b", b=B)
            nfyts = [cpool.tile([128, C * I1], f16, tag=f"nfy{t}",
                                name=f"nfy{t}") for t in range(NT)]
            fwxts = [cpool.tile([128, C * XS], f16, tag=f"fwx{t}",
                                name=f"fwx{t}") for t in range(NT)]
            nfc1ts = [cpool.tile([128, 2 * PAIRS], f16, tag=f"nfc1{t}",
                                 name=f"nfc1{t}") for t in range(NT)]
            ybufs = [cpool.tile([128, C * XS], f32, tag=f"yb{t}",
                                name=f"yb{t}") for t in range(NT)]

            def emit_resident_loads():
                # nfa: only the populated partition rows, split for queue
                # parallelism (pad rows are never read by the matmuls)
                for par in (0, 1):
                    r0 = 64 * par
                    for s in range(4):
                        lo = (I1 * s) // 4
                        hi = (I1 * (s + 1)) // 4
                        nc.scalar.dma_start(
                            out=nfasb[r0 + lo:r0 + hi, :],
                            in_=nfa[r0 + lo:r0 + hi, :])
                for t in range(NT):
                    nc.scalar.dma_start(out=nfyts[t][:, :],
                                        in_=nfy[t * 128:(t + 1) * 128, :])
                    nc.scalar.dma_start(out=fwxts[t][:, :],
                                        in_=fwx[t * 128:(t + 1) * 128, :])
                    nc.scalar.dma_start(out=nfc1ts[t][:, :],
                                        in_=nfc1[t * 128:(t + 1) * 128, :])

            # ---- aug build: [96, 294] = w21.T @ u2aug; halves go to the
            # i'=48 row of each half's w3u block ----
            aps = psum.tile([128, MCHUNK], f32, tag="z")
            nc.tensor.matmul(aps[:C, :NX], w21sb[:27, :], u2sb[:27, :],
                             start=True, stop=True)
            astg = stgpool.tile([C, MCHUNK], f16, tag="stg")
            nc.scalar.copy(astg[:, :NX], aps[:C, :NX])
            for hh in range(2):
                nc.sync.dma_start(
                    out=w3u_scr[:, hh * I1 * NXH + I * NXH:
                                hh * I1 * NXH + I1 * NXH],
                    in_=astg[:, hh * NXH:(hh + 1) * NXH])

            # per-half w3u view: [c, h, i', (xl,y')]
            w3u_v = w3u_scr[:, :].rearrange("c (h i xy) -> c h i xy",
                                            h=2, xy=NXH)
            if debug == "A":
                npairs = 0
            elif isinstance(debug, int):
                npairs = debug
            else:
                npairs = PAIRS

            def emit_group(hh, cp, t, lts):
                """One phase-B unit: Z matmuls + fused MAC-scan + extract."""
                c0, c1 = 2 * cp, 2 * cp + 1
                nfyv = nfyts[t][:, c0 * I1:(c1 + 1) * I1].rearrange(
                    "p (c i) -> p c i", i=I1)
                rep = reppool.tile([128, 2 * NXH], f16, tag="rep")
                rv = rep[:, :].rearrange("p (c x y) -> p c x y",
                                         c=2, y=Y1)
                nfyb = nfyv[:, :, None, :].to_broadcast([128, 2, XSH, Y1])
                nc.scalar.copy(rv, nfyb)
                zt = zpsum.tile([128, 1024], f32, tag="zz")
                lt = lts[cp]
                for ci in range(2):
                    lhsT = nfav[64 * ci:64 * ci + I1, cp,
                                t * 128:(t + 1) * 128]
                    nc.tensor.matmul(zt[:, 512 * ci:512 * ci + NXH], lhsT,
                                     lt[64 * ci:64 * ci + I1, :],
                                     start=True, stop=True)
                # fused multiply+prefix-scan over both c's; the c0->c1
                # carry is cancelled via nfc1 in the final pass
                sc = scpool.tile([128, 2 * NXH], f32, tag="sc")
                zv = zt[:, :].rearrange("p (c n) -> p c n", n=512)[:, :, 0:NXH]
                rv2 = rep[:, :].rearrange("p (c n) -> p c n", n=NXH)
                ov = sc[:, :].rearrange("p (c n) -> p c n", n=NXH)
                nc.vector._custom_dve(mac_scan, out=ov, in0=zv, in1=rv2)
                bnd = sc[:, :].rearrange(
                    "p (c x y) -> p c x y", c=2, y=Y1)[:, :, :, I]
                ybv = ybufs[t][:, cp * 2 * XS:(cp + 1) * 2 * XS].rearrange(
                    "p (c x) -> p c x", x=XS)[:, :, 3 * hh:3 * hh + 3]
                nc.scalar.copy(ybv, bnd)

            def emit_lt(hh, cp):
                c0, c1 = 2 * cp, 2 * cp + 1
                lt = ltpool.tile([128, NXH], f16, tag="lt")
                leng = nc.sync if (cp % 2 == 0) else nc.scalar
                leng.dma_start(out=lt[0:I1, :], in_=w3u_v[c0, hh])
                leng.dma_start(out=lt[64:64 + I1, :], in_=w3u_v[c1, hh])
                return lt

            # phase-B group emission state for interleaving into the build
            groups = {0: [(cp, t) for cp in range(npairs)
                          for t in range(NT)],
                      1: [(cp, t) for cp in range(npairs)
                          for t in range(NT)]}
            lts = {0: {}, 1: {}}
            emitted = {0: 0, 1: 0}

            def emit_groups(hh, n):
                g = groups[hh]
                while emitted[hh] < len(g) and n > 0:
                    cp, t = g[emitted[hh]]
                    if t == 0:
                        lts[hh][cp] = emit_lt(hh, cp)
                    emit_group(hh, cp, t, lts[hh])
                    emitted[hh] += 1
                    n -= 1

            # ---- W3U build per x-half, k-accumulated; build of half 1
            # overlaps phase B of half 0 (groups interleaved into the kt
            # stream so PE's in-order queue alternates between them) ----
            NMCH = (MPH + 2 * MCHUNK - 1) // (2 * MCHUNK)  # 4
            for hh in range(2):
                for mcp in range(NMCH):
                    w2c = min(2 * MCHUNK, MPH - mcp * 2 * MCHUNK)
                    wa = min(MCHUNK, w2c)
                    wb = w2c - wa
                    psa = psum.tile([128, MCHUNK], f32, tag="z",
                                    name=f"bpa{hh}_{mcp}")
                    psb = (psum.tile([128, MCHUNK], f32, tag="z",
                                     name=f"bpb{hh}_{mcp}") if wb > 0
                           else None)
                    for kt in range(KT):
                        t = u3pool.tile([128, 2 * MCHUNK], f16, tag="u3")
                        eng = nc.sync if (kt % 2 == 0) else nc.scalar
                        base = hh * MPH + mcp * 2 * MCHUNK
                        eng.dma_start(
                            out=t[:, :w2c],
                            in_=u3t[kt * 128:(kt + 1) * 128,
                                    base:base + w2c])
                        for off in range(0, wa, 512):
                            h = min(512, wa - off)
                            nc.tensor.matmul(
                                psa[:C, off:off + h], w3v[:, kt, :],
                                t[:, off:off + h],
                                start=(kt == 0), stop=(kt == KT - 1))
                        for off in range(0, wb, 512):
                            h = min(512, wb - off)
                            nc.tensor.matmul(
                                psb[:C, off:off + h], w3v[:, kt, :],
                                t[:, MCHUNK + off:MCHUNK + off + h],
                                start=(kt == 0), stop=(kt == KT - 1))
                        if hh == 1 and INTERLEAVE:
                            emit_groups(0, 5)
                    if hh == 0 and mcp == 1:
                        emit_resident_loads()
                    for half, ps, wh in ((0, psa, wa), (1, psb, wb)):
                        if wh <= 0:
                            continue
                        stg = stgpool.tile([C, MCHUNK], f16, tag="stg")
                        nc.scalar.copy(stg[:, :wh], ps[:C, :wh])
                        base = hh * I1 * NXH + (2 * mcp + half) * MCHUNK
                        nc.sync.dma_start(
                            out=w3u_scr[:, base:base + wh],
                            in_=stg[:, :wh])

            # remaining phase-B groups (half 0 leftovers, then half 1)
            emit_groups(0, 10 ** 9)
            emit_groups(1, 10 ** 9)
            if debug != "A":
                for t in range(NT):
                    dnv = fwxts[t][:, :].rearrange("p (c x) -> p c x", x=XS)
                    ybv = ybufs[t][:, :].rearrange("p (c x) -> p c x", x=XS)
                    yn = finpool.tile([128, C * XS], f32, tag="yn")
                    ynv = yn[:, :].rearrange("p (c x) -> p c x", x=XS)
                    nc.vector.tensor_tensor(ynv, ybv, dnv, mult)
                    ostf = finpool.tile([128, C], f32, tag="ostf")
                    nc.vector.tensor_reduce(
                        ostf[:, :], ynv, axis=mybir.AxisListType.X, op=add)
                    # cancel each half-scan's c0->c1 carry:
                    # ost[c1] -= T0_h * nf[c1, x_{h,0}] with T0_h =
                    # bnd[c0, xl=2] of half h (the full c0 half-row sum)
                    ostodd = ostf[:, :].rearrange(
                        "p (cp c) -> p cp c", c=2)[:, :, 1]
                    for hh in range(2):
                        t0v = ybufs[t][:, :].rearrange(
                            "p (cp cx) -> p cp cx",
                            cx=2 * XS)[:, :, 3 * hh + 2]
                        corr = finpool.tile([128, PAIRS], f32,
                                            tag=f"corr{hh}")
                        nc.vector.tensor_tensor(
                            corr[:, :], t0v,
                            nfc1ts[t][:, hh * PAIRS:(hh + 1) * PAIRS], mult)
                        nc.vector.tensor_tensor(ostodd, ostodd, corr[:, :],
                                                mybir.AluOpType.subtract)
                    nc.sync.dma_start(out=outp[t * 128:(t + 1) * 128, :],
                                      in_=ostf[:, :])
    nc.finalize()
    return nc


def _prep_inputs(node_feats, w3, w2, w1, U3, U2, U1):
    """Host-side sharding / re-layout: transposes, dtype casts, padding,
    concatenation, and the Abel-summation difference of adjacent nf_x."""
    f16 = np.float16
    f32 = np.float32
    node_feats = np.asarray(node_feats, dtype=f32)
    nf16 = node_feats.astype(f16)

    # shared across cores
    w3p = np.zeros((KP, C), dtype=f16)
    w3p[:K3] = np.asarray(w3, dtype=f32).astype(f16)
    w21 = np.zeros((32, C), dtype=f16)
    w21[:K2] = np.asarray(w2, dtype=f32).astype(f16)
    w21[K2:K2 + K1] = np.asarray(w1, dtype=f32).astype(f16)

    # nfa: [p, cp, b]; p = 64*(c%2) + i'; i'=48 row is the ones channel
    nfT = nf16.transpose(1, 2, 0)  # [c, i, b]
    nfa = np.zeros((128, PAIRS, B), dtype=f16)
    for par in (0, 1):
        nfa[64 * par:64 * par + I] = nfT[par::2].transpose(1, 0, 2)
        nfa[64 * par + I] = 1.0
    nfa = np.ascontiguousarray(nfa.reshape(128, PAIRS * B))

    # nfy: [b, c, 49] = nf with ones channel
    nfy = np.empty((B, C, I1), dtype=f16)
    nfy[:, :, :I] = nf16
    nfy[:, :, I] = 1.0
    nfy = np.ascontiguousarray(nfy.reshape(B, C * I1))

    # One shared fp16 transpose of U3 to [k, i, x, y], then per-core
    # x-slice + y-pad + k-pad.
    U3_16 = np.asarray(U3, dtype=f32).astype(f16)
    u3_kixy = np.ascontiguousarray(U3_16.transpose(3, 2, 0, 1))  # [k,i,x,y]
    U2_16 = np.asarray(U2, dtype=f32).astype(f16)
    U1_16 = np.asarray(U1, dtype=f32).astype(f16)

    in_maps = []
    for r in range(NCORES):
        xlo = XS * r
        u3a = np.zeros((KP, I, XS, Y1), dtype=f16)
        u3a[:K3, :, :, :I] = u3_kixy[:, :, xlo:xlo + XS, :]
        # m-order (h, i, xl, y') so each x-half is a contiguous build block
        u3t = np.ascontiguousarray(
            u3a.reshape(KP, I, 2, XSH, Y1).transpose(0, 2, 1, 3, 4)
            .reshape(KP, MP))

        # u2aug: rows 0:24 U2 slice, rows 24:27 U1 slice (at y'=48)
        u2a = np.zeros((32, XS, Y1), dtype=f16)
        u2a[:K2, :, :I] = U2_16[xlo:xlo + XS].transpose(2, 0, 1)
        u2a[K2:K2 + K1, :, I] = U1_16[xlo:xlo + XS].T
        u2a = np.ascontiguousarray(u2a.reshape(32, NX))

        # Abel-summation weights per x-half: device wrote within-half
        # boundary cums, so out_c = sum_h sum_xl bnd[h,xl]*dnfx[h,xl] with
        # dnfx = nf[x] - nf[x+1] inside each half (last xl keeps nf[x]).
        sl = node_feats[:, :, xlo:xlo + XS]
        fwx = np.empty((B, C, 2, XSH), dtype=f32)
        slh = sl.reshape(B, C, 2, XSH)
        fwx[:, :, :, :XSH - 1] = slh[:, :, :, :XSH - 1] - slh[:, :, :, 1:]
        fwx[:, :, :, XSH - 1] = slh[:, :, :, XSH - 1]
        fwx = np.ascontiguousarray(fwx.astype(f16).reshape(B, C * XS))
        nfc1 = np.empty((B, 2, PAIRS), dtype=f32)
        for hh in range(2):
            nfc1[:, hh, :] = node_feats[:, 1::2, xlo + 3 * hh]
        nfc1 = np.ascontiguousarray(nfc1.astype(f16).reshape(B, 2 * PAIRS))

        in_maps.append({
            "u3t": u3t,
            "w3p": w3p,
            "nfa": nfa,
            "nfy": nfy,
            "fwx": fwx,
            "nfc1": nfc1,
            "u2aug": u2a,
            "w21": w21,
        })
    return in_maps


def kernel(node_feats, w3, w2, w1, U3, U2, U1):
    global LAST_EXEC_NS
    import os
    from concourse.bass_utils import run_bass_kernel_spmd

    if "nc" not in _CACHE:
        _CACHE["nc"] = _build_nc()
    nc = _CACHE["nc"]

    in_maps = _prep_inputs(node_feats, w3, w2, w1, U3, U2, U1)
    trace = bool(os.environ.get("BASS_TRACE"))
    res = run_bass_kernel_spmd(nc, in_maps, list(range(NCORES)), trace=trace)
    LAST_EXEC_NS = res.exec_time_ns
    _CACHE["last_results"] = res

    out = np.zeros((B, C), dtype=np.float64)
    for r in range(NCORES):
        out += res.results[r]["out"].astype(np.float64)
    return out.astype(np.float32)



# revision 3
# speedup vs baseline: 2.0870x; 2.0870x over previous
"""Trainium2 Bass kernel for the MACE-style symmetric contraction:

    out  = einsum("xyik,kc,bci->bcxy", U3, w3, nf)
    c2   = einsum("xyk,kc->cxy", U2, w2)[None] + out
    out  = einsum("bcxi,bci->bcx", c2, nf)
    c1   = einsum("xk,kc->cx", U1, w1)[None] + out
    out  = einsum("bci,bci->bc", c1, nf)

Algebraically:

    out[b,c] =   sum_{x,y,i} W3U[x,y,i,c] nf[b,c,x] nf[b,c,y] nf[b,c,i]
               + sum_{x,y}   U2w2[c,x,y]  nf[b,c,x] nf[b,c,y]
               + sum_{x}     U1w1[c,x]    nf[b,c,x]

with W3U = einsum("xyik,kc->xyic", U3, w3).  Since nf_x*nf_y is symmetric
in (x,y), only the (x,y)-symmetric part of W3U/U2w2 contributes: fold the
rectangle onto unordered pairs {X, y<=X} via SYM[X,y] = W3U[X,y]+W3U[y,X]
(diagonal halved).  This halves the U3 HBM stream, the build matmul
columns, and the phase-B work vs the unfolded form.  The U1 term is added
on the host (tiny).

Sharding: each core owns 6 X-values {r, 15-r, 16+r, 31-r, 32+r, 47-r},
paired into 3 fold groups (Xa, Xb=47-Xa).  A group's 49 columns are
[Xa-run: y=0..Xa][Xb-run: y=0..Xb] - rectangular across cores, so one
SPMD program serves all cores; per-core structure lives in the data.

Device pipeline (fp16 data, fp32 PSUM/scan state):
  build:   A2[c, i', (g,w)] = w3.T @ u3s on PE, k-accumulated in PSUM,
           drained fp16 to a DRAM scratch (i'=48 row carries folded U2w2,
           contracted against a ones channel in nfa).
  phase B: per (c-pair, b-tile): Z[b,(g,w)] = nfa.T @ A2_c on PE; a fused
           DVE MAC-scan against the host-streamed weight tensor
           nfprod[b,c,w] = nf_y(w)*nf_X(w) accumulates Z*nfprod, so the
           scan value at the end of each c's 147 columns IS out[b,c]
           (c1 = end minus c0's end, handled in the final pass).
"""

import numpy as np

B = 512          # atoms
C = 96           # feats
I = 48           # irreps
K3, K2, K1 = 1270, 24, 3
NCORES = 8
I1 = I + 1        # 49 contraction rows (i + U2 aug row)
W = 49            # folded group width
G = 3             # fold groups per core
NW = G * W        # 147 columns per core
MP = I * NW       # 7056 build m-columns (m = i*NW + g*W + w)
SCR = I1 * NW     # 7203 scratch cols per c (aug row at 7056..7202)
KP = 1280         # K3 padded to 10 partition tiles
KT = KP // 128    # 10
MCHUNK = 1024
PAIRS = C // 2    # 48
NT = B // 128     # 4 b-tiles
NBLK = PAIRS // 4  # 12 lt blocks of 4 c-pairs

_CACHE = {}

# exec time of the last device run (ns), when BASS_TRACE=1
LAST_EXEC_NS = None


def _core_pairs(r):
    """Fold pairs (Xa, Xb) with Xa+Xb=47; Xa-run first (y=0..Xa)."""
    return [(r, 47 - r), (15 - r, 32 + r), (16 + r, 31 - r)]


def _register_mac_scan():
    """Custom DVE op: out[t] = prefix-sum of in0[t]*in1[t] (fp32 state)."""
    import concourse.dve_ops as dve_ops_mod
    if any(op.name == "TT_MAC_SCAN_ANT" for op in dve_ops_mod.OPS):
        return next(op for op in dve_ops_mod.OPS
                    if op.name == "TT_MAC_SCAN_ANT")
    from concourse.dve_spec import Spec, scan, Src0, Src1
    from concourse.dve_uop import AluOp
    from concourse.dve_ops import DveOp

    def _ref_mac_scan(in0, in1, s0, s1, imm2):
        p = in0.astype(np.float32) * in1.astype(np.float32)
        return np.cumsum(p.reshape(p.shape[0], -1), axis=1).reshape(
            p.shape).astype(np.float32)

    spec = Spec(body=scan(AluOp.ADD, Src0 * Src1), reference=_ref_mac_scan)
    op = DveOp("TT_MAC_SCAN_ANT", spec, subdim=False,
               uops_sha={"v3": "b3fc3e78a862b7eb",
                         "v4": "bc6a002865d48b97"})
    dve_ops_mod.OPS.append(op)
    dve_ops_mod.CUSTOM_DVE_SPECS[op.name] = spec
    dve_ops_mod._SUB_OPCODE_FOR_NAME[op.name] = (
        max(dve_ops_mod._SUB_OPCODE_FOR_NAME.values()) + 1)
    return op


def _build_nc(debug=None):
    import concourse.mybir as mybir
    from concourse.tile import TileContext

    mac_scan = _register_mac_scan()

    f16 = mybir.dt.float16
    f32 = mybir.dt.float32
    mult = mybir.AluOpType.mult
    sub = mybir.AluOpType.subtract

    import concourse.bacc as bacc
    nc = bacc.Bacc(None, target_bir_lowering=False)
    u3t = nc.dram_tensor("u3t", [KP, MP], f16, kind="ExternalInput")
    # w3p columns are c-reordered host-side: [even c's | odd c's], so the
    # build PSUM rows land even-c in rows 0..47, odd-c in rows 48..95 and
    # lt loads need no strided row access.
    w3p = nc.dram_tensor("w3p", [KP, C], f16, kind="ExternalInput")
    # nfa[p, t, cp, b128]: p = 64*(c%2) + i'; i'=48 row is ones
    nfa = nc.dram_tensor("nfa", [128, NT * PAIRS * 128], f16,
                         kind="ExternalInput")
    # nfprod[b, c*NW]: per-column weight nf_y(w)*nf_X(w)
    nfprod = nc.dram_tensor("nfprod", [B, C * NW], f16,
                            kind="ExternalInput")
    u2aug = nc.dram_tensor("u2aug", [32, NW], f16, kind="ExternalInput")
    w21 = nc.dram_tensor("w21", [32, C], f16, kind="ExternalInput")
    outp = nc.dram_tensor("out", [B, C], f32, kind="ExternalOutput")

    with TileContext(nc) as tc:
        with (
            nc.allow_low_precision(reason="fp16 intermediates; rel-err "
                                   "budget 2e-2 vs ~1e-3 incurred"),
            tc.tile_pool(name="dram", bufs=1, space="DRAM") as dpool,
            tc.tile_pool(name="const", bufs=1) as cpool,
            tc.tile_pool(name="u3", bufs=4) as u3pool,
            tc.tile_pool(name="psum", bufs=2, space="PSUM") as psum,
            tc.tile_pool(name="zpsum", bufs=2, space="PSUM") as zpsum,
            tc.tile_pool(name="lt", bufs=1) as ltpool,
            tc.tile_pool(name="sc", bufs=2) as scpool,
            tc.tile_pool(name="nfr", bufs=2) as nfrpool,
            tc.tile_pool(name="nfat", bufs=2) as nfapool,
            tc.tile_pool(name="stg", bufs=3) as stgpool,
            tc.tile_pool(name="fin", bufs=2) as finpool,
        ):
            # scratch row c = [i-major build cols 0..7055 | aug 7056..7202]
            # rows: even c's at 0..47, odd at 48..95 (w3p reorder)
            w3u_scr = dpool.tile([C, SCR], f16)

            # ---- u3 stream + build own the sync queue; residents go on
            # the scalar queue ----
            w3sb = cpool.tile([128, KT * C], f16)
            w3v = w3sb[:, :].rearrange("p (k c) -> p k c", c=C)
            nc.sync.dma_start(
                out=w3v[:, :, :],
                in_=w3p[:, :].rearrange("(k p) c -> p k c", p=128))
            w21sb = cpool.tile([32, C], f16)
            nc.sync.dma_start(out=w21sb[:, :], in_=w21[:, :])
            u2sb = cpool.tile([32, NW], f16)
            nc.sync.dma_start(out=u2sb[:, :], in_=u2aug[:, :])

            nfav = nfa[:, :].rearrange("p (t m) -> p t m", t=NT)

            def load_nfa(t):
                nt = nfapool.tile([128, PAIRS * 128], f16, tag="nfa")
                for par in (0, 1):
                    r0 = 64 * par
                    nc.scalar.dma_start(out=nt[r0:r0 + I1, :],
                                        in_=nfav[r0:r0 + I1, t, :])
                return nt

            def load_nfprod(t):
                nt = nfrpool.tile([128, C * NW], f16, tag="nfr")
                nc.scalar.dma_start(out=nt[:, :],
                                    in_=nfprod[t * 128:(t + 1) * 128, :])
                return nt

            nfa_t = load_nfa(0)
            nfp_t = load_nfprod(0)

            # ---- aug build: [96, 147] = w21.T @ u2aug -> aug row ----
            aps = psum.tile([128, MCHUNK], f32, tag="z")
            nc.tensor.matmul(aps[:C, :NW], w21sb[:K2, :], u2sb[:K2, :],
                             start=True, stop=True)
            astg = stgpool.tile([C, MCHUNK], f16, tag="stg")
            nc.scalar.copy(astg[:, :NW], aps[:C, :NW])
            nc.sync.dma_start(out=w3u_scr[:, MP:SCR], in_=astg[:, :NW])

            # ---- build: A2[c, m] = w3.T @ u3s, k-accumulated ----
            NMCH = (MP + 2 * MCHUNK - 1) // (2 * MCHUNK)  # 4
            for mcp in range(NMCH):
                w2c = min(2 * MCHUNK, MP - mcp * 2 * MCHUNK)
                wa = min(MCHUNK, w2c)
                wb = w2c - wa
                psa = psum.tile([128, MCHUNK], f32, tag="z",
                                name=f"bpa{mcp}")
                psb = (psum.tile([128, MCHUNK], f32, tag="z",
                                 name=f"bpb{mcp}") if wb > 0 else None)
                for kt in range(KT):
                    tl = u3pool.tile([128, 2 * MCHUNK], f16, tag="u3")
                    base = mcp * 2 * MCHUNK
                    nc.sync.dma_start(
                        out=tl[:, :w2c],
                        in_=u3t[kt * 128:(kt + 1) * 128, base:base + w2c])
                    for off in range(0, wa, 512):
                        h = min(512, wa - off)
                        nc.tensor.matmul(
                            psa[:C, off:off + h], w3v[:, kt, :],
                            tl[:, off:off + h],
                            start=(kt == 0), stop=(kt == KT - 1))
                    for off in range(0, wb, 512):
                        h = min(512, wb - off)
                        nc.tensor.matmul(
                            psb[:C, off:off + h], w3v[:, kt, :],
                            tl[:, MCHUNK + off:MCHUNK + off + h],
                            start=(kt == 0), stop=(kt == KT - 1))
                for half, ps, wh in ((0, psa, wa), (1, psb, wb)):
                    if wh <= 0:
                        continue
                    stg = stgpool.tile([C, MCHUNK], f16, tag="stg")
                    nc.scalar.copy(stg[:, :wh], ps[:C, :wh])
                    base = mcp * 2 * MCHUNK + half * MCHUNK
                    nc.sync.dma_start(
                        out=w3u_scr[:, base:base + wh], in_=stg[:, :wh])

            # ---- lt loads: 12 blocks x 2 DMAs (even-c c0's to tile rows
            # 0..48, odd-c c1's to rows 64..112) ----
            lts = []
            scr_v = w3u_scr[:, :]
            for blk in range(NBLK):
                lt = ltpool.tile([128, 4 * NW], f16, tag=f"lt{blk}",
                                 name=f"lt{blk}")
                ltv = lt[:, :].rearrange("p (c m) -> p c m", m=NW)
                for par in (0, 1):
                    rows = scr_v[48 * par + 4 * blk:48 * par + 4 * blk + 4,
                                 :].rearrange("c (i m) -> i c m", m=NW)
                    eng = nc.sync if (blk % 2 == 0) else nc.scalar
                    eng.dma_start(out=ltv[64 * par:64 * par + I1, :, :],
                                  in_=rows)
                lts.append(lt)

            # ---- phase B, t-major ----
            npairs = PAIRS if debug is None else debug
            for t in range(NT):
                if t + 1 < NT:
                    nfa_next = load_nfa(t + 1)
                    nfp_next = load_nfprod(t + 1)
                sct = scpool.tile([128, PAIRS * 2 * NW], f16, tag="sc")
                nfp_v = nfp_t[:, :].rearrange("p (c m) -> p c m", m=NW)
                for cp in range(npairs):
                    blk, j = divmod(cp, 4)
                    lt = lts[blk]
                    zt = zpsum.tile([128, 1024], f32, tag="zz")
                    for ci in range(2):
                        lhsT = nfa_t[64 * ci:64 * ci + I1,
                                     cp * 128:(cp + 1) * 128]
                        rhs = lt[64 * ci:64 * ci + I1,
                                 j * NW:(j + 1) * NW]
                        nc.tensor.matmul(zt[:, 512 * ci:512 * ci + NW],
                                         lhsT, rhs, start=True, stop=True)
                    zv = zt[:, :].rearrange(
                        "p (c n) -> p c n", n=512)[:, :, 0:NW]
                    rv = nfp_v[:, 2 * cp:2 * cp + 2, :]
                    ov = sct[:, cp * 2 * NW:(cp + 1) * 2 * NW].rearrange(
                        "p (c n) -> p c n", n=NW)
                    nc.vector._custom_dve(mac_scan, out=ov, in0=zv, in1=rv)

                # final: out_c0 = S[end of c0] ; out_c1 = S[end] - S[mid]
                scv = sct[:, :].rearrange("p (cp c m) -> p cp c m",
                                          c=2, m=NW)
                ostf = finpool.tile([128, C], f32, tag="ostf")
                ostv = ostf[:, :].rearrange("p (cp c) -> p cp c", c=2)
                nc.vector.tensor_copy(ostv[:, :, 0:1],
                                      scv[:, :, 0, NW - 1:NW])
                nc.vector.tensor_tensor(ostv[:, :, 1:2],
                                        scv[:, :, 1, NW - 1:NW],
                                        scv[:, :, 0, NW - 1:NW], sub)
                nc.sync.dma_start(out=outp[t * 128:(t + 1) * 128, :],
                                  in_=ostf[:, :])
                if t + 1 < NT:
                    nfa_t = nfa_next
                    nfp_t = nfp_next
    nc.finalize()
    return nc


def _prep_inputs(node_feats, w3, w2, w1, U3, U2, U1):
    """Host-side fold, re-layout, fp16 casts, per-core sharding."""
    f16 = np.float16
    f32 = np.float32
    node_feats = np.asarray(node_feats, dtype=f32)
    nf16 = node_feats.astype(f16)

    # c-reorder for the build PSUM rows: [even c's | odd c's]
    c_perm = np.concatenate([np.arange(0, C, 2), np.arange(1, C, 2)])
    w3p = np.zeros((KP, C), dtype=f16)
    w3p[:K3] = np.asarray(w3, dtype=f32).astype(f16)[:, c_perm]
    w21 = np.zeros((32, C), dtype=f16)
    w21[:K2] = np.asarray(w2, dtype=f32).astype(f16)[:, c_perm]

    # nfa[p, t, cp, b]: p = 64*(c%2) + i'; i'=48 row is ones
    nfT = nf16.transpose(1, 2, 0)  # [c, i, b]
    nfa = np.zeros((128, NT, PAIRS, 128), dtype=f16)
    for par in (0, 1):
        s = nfT[par::2].transpose(1, 0, 2).reshape(I, PAIRS, NT, 128)
        nfa[64 * par:64 * par + I] = s.transpose(0, 2, 1, 3)
        nfa[64 * par + I] = 1.0
    nfa = np.ascontiguousarray(nfa.reshape(128, NT * PAIRS * 128))

    # fold U3: SYM[k, i, X, y] = U3w3-src folded over (x,y); diag halved
    U3_32 = np.asarray(U3, dtype=f32)
    u3_kixy = np.ascontiguousarray(U3_32.transpose(3, 2, 0, 1))
    SYM = (u3_kixy + u3_kixy.transpose(0, 1, 3, 2)).astype(f16)
    del u3_kixy
    U2f = np.asarray(U2, dtype=f32).transpose(2, 0, 1)
    U2S = (U2f + U2f.transpose(0, 2, 1)).astype(f16)

    # host U1 term (tiny): out1[b, c] = sum_x U1w1[c,x] nf[b,c,x]
    U1w1 = np.einsum("xk,kc->cx", np.asarray(U1, f32), np.asarray(w1, f32))
    host_out = np.einsum("cx,bcx->bc", U1w1,
                         node_feats.astype(np.float64))

    in_maps = []
    for r in range(NCORES):
        pairs = _core_pairs(r)
        u3a = np.zeros((KP, I, NW), dtype=f16)
        u2a = np.zeros((32, NW), dtype=f16)
        yidx = np.zeros(NW, dtype=np.int64)
        xidx = np.zeros(NW, dtype=np.int64)
        for g, (xa, xb) in enumerate(pairs):
            u3a[:K3, :, g * W:g * W + xa + 1] = SYM[:, :, xa, 0:xa + 1]
            u3a[:K3, :, g * W + xa] = SYM[:, :, xa, xa] / 2
            u2a[:K2, g * W:g * W + xa + 1] = U2S[:, xa, 0:xa + 1]
            u2a[:K2, g * W + xa] = U2S[:, xa, xa] / 2
            yidx[g * W:g * W + xa + 1] = np.arange(xa + 1)
            xidx[g * W:g * W + xa + 1] = xa
            u3a[:K3, :, g * W + xa + 1:g * W + W] = SYM[:, :, xb, 0:xb + 1]
            u3a[:K3, :, g * W + 48] = SYM[:, :, xb, xb] / 2
            u2a[:K2, g * W + xa + 1:g * W + W] = U2S[:, xb, 0:xb + 1]
            u2a[:K2, g * W + 48] = U2S[:, xb, xb] / 2
            yidx[g * W + xa + 1:g * W + W] = np.arange(xb + 1)
            xidx[g * W + xa + 1:g * W + W] = xb
        u3t = np.ascontiguousarray(u3a.reshape(KP, MP))

        # nfprod[b, c, w] = nf_y(w) * nf_X(w), fp32 product cast to fp16
        nfprod = np.ascontiguousarray(
            (node_feats[:, :, yidx] * node_feats[:, :, xidx])
            .astype(f16).reshape(B, C * NW))

        in_maps.append({
            "u3t": u3t,
            "w3p": w3p,
            "nfa": nfa,
            "nfprod": nfprod,
            "u2aug": u2a,
            "w21": w21,
        })
    return in_maps, host_out


def kernel(node_feats, w3, w2, w1, U3, U2, U1):
    global LAST_EXEC_NS
    import os
    from concourse.bass_utils import run_bass_kernel_spmd

    if "nc" not in _CACHE:
        _CACHE["nc"] = _build_nc()
    nc = _CACHE["nc"]

    in_maps, host_out = _prep_inputs(node_feats, w3, w2, w1, U3, U2, U1)
    trace = bool(os.environ.get("BASS_TRACE"))
    res = run_bass_kernel_spmd(nc, in_maps, list(range(NCORES)),
                               trace=trace)
    LAST_EXEC_NS = res.exec_time_ns
    _CACHE["last_results"] = res

    out = host_out.copy()
    for r in range(NCORES):
        out += res.results[r]["out"].astype(np.float64)
    return out.astype(np.float32)


# revision 6
# speedup vs baseline: 2.4224x; 1.1607x over previous
"""Trainium2 Bass kernel for the MACE-style symmetric contraction:

    out  = einsum("xyik,kc,bci->bcxy", U3, w3, nf)
    c2   = einsum("xyk,kc->cxy", U2, w2)[None] + out
    out  = einsum("bcxi,bci->bcx", c2, nf)
    c1   = einsum("xk,kc->cx", U1, w1)[None] + out
    out  = einsum("bci,bci->bc", c1, nf)

Algebraically:

    out[b,c] =   sum_{x,y,i} W3U[x,y,i,c] nf[b,c,x] nf[b,c,y] nf[b,c,i]
               + sum_{x,y}   U2w2[c,x,y]  nf[b,c,x] nf[b,c,y]
               + sum_{x}     U1w1[c,x]    nf[b,c,x]

with W3U = einsum("xyik,kc->xyic", U3, w3).  Since nf_x*nf_y is symmetric
in (x,y), only the (x,y)-symmetric part of W3U/U2w2 contributes: fold the
rectangle onto unordered pairs {X, y<=X} via SYM[X,y] = W3U[X,y]+W3U[y,X]
(diagonal halved).  This halves the U3 HBM stream, the build matmul
columns, and the phase-B work vs the unfolded form.  The U1 term is added
on the host (tiny).

Sharding: each core owns 6 X-values {r, 15-r, 16+r, 31-r, 32+r, 47-r},
paired into 3 fold groups (Xa, Xb=47-Xa).  A group's 49 columns are
[Xa-run: y=0..Xa][Xb-run: y=0..Xb] - rectangular across cores, so one
SPMD program serves all cores; per-core structure lives in the data.

Device pipeline (fp16 data, fp32 PSUM/scan state):
  build:   A2[c, i', (g,w)] = w3.T @ u3s on PE, k-accumulated in PSUM,
           drained fp16 to a DRAM scratch (i'=48 row carries folded U2w2,
           contracted against a ones channel in nfa).  Scratch writes go
           out on the (otherwise idle) gpsimd SWDGE queue so they never
           block the sync queue's u3 stream.
  phase B: per (4-c block, b-tile): Z[b,(g,w)] = nfa.T @ A2_c on PE into
           a [128,2048] PSUM tile (4 c's); one fused DVE MAC-scan against
           the host-streamed weight tensor nfprod[b,c,w] = nf_y(w)*nf_X(w)
           accumulates Z*nfprod, so the scan value at the end of each c's
           147 columns IS out[b,c] (recovered by a shifted subtract).
"""

import numpy as np

B = 512          # atoms
C = 96           # feats
I = 48           # irreps
K3, K2, K1 = 1270, 24, 3
NCORES = 8
I1 = I + 1        # 49 contraction rows (i + U2 aug row)
W = 49            # folded group width
G = 3             # fold groups per core
NW = G * W        # 147 columns per core
MP = I * NW       # 7056 build m-columns (m = i*NW + g*W + w)
SCR = I1 * NW     # 7203 scratch cols per c (aug row at 7056..7202)
KP = 1280         # K3 padded to 10 partition tiles
KT = KP // 128    # 10
MCHUNK = 2048     # build chunk (one PSUM tile, 4 banks)
PAIRS = C // 2    # 48
NT = B // 128     # 4 b-tiles
NQ = C // 4       # 24 phase-B units per b-tile (4 c's each)

_CACHE = {}

# exec time of the last device run (ns), when BASS_TRACE=1
LAST_EXEC_NS = None


def _core_pairs(r):
    """Fold pairs (Xa, Xb) with Xa+Xb=47; Xa-run first (y=0..Xa)."""
    return [(r, 47 - r), (15 - r, 32 + r), (16 + r, 31 - r)]


def _register_mac_scan():
    """Custom DVE op: out[t] = prefix-sum of in0[t]*in1[t] (fp32 state)."""
    import concourse.dve_ops as dve_ops_mod
    if any(op.name == "TT_MAC_SCAN_ANT" for op in dve_ops_mod.OPS):
        return next(op for op in dve_ops_mod.OPS
                    if op.name == "TT_MAC_SCAN_ANT")
    from concourse.dve_spec import Spec, scan, Src0, Src1
    from concourse.dve_uop import AluOp
    from concourse.dve_ops import DveOp

    def _ref_mac_scan(in0, in1, s0, s1, imm2):
        p = in0.astype(np.float32) * in1.astype(np.float32)
        return np.cumsum(p.reshape(p.shape[0], -1), axis=1).reshape(
            p.shape).astype(np.float32)

    spec = Spec(body=scan(AluOp.ADD, Src0 * Src1), reference=_ref_mac_scan)
    op = DveOp("TT_MAC_SCAN_ANT", spec, subdim=False,
               uops_sha={"v3": "b3fc3e78a862b7eb",
                         "v4": "bc6a002865d48b97"})
    dve_ops_mod.OPS.append(op)
    dve_ops_mod.CUSTOM_DVE_SPECS[op.name] = spec
    dve_ops_mod._SUB_OPCODE_FOR_NAME[op.name] = (
        max(dve_ops_mod._SUB_OPCODE_FOR_NAME.values()) + 1)
    return op


def _build_nc(debug=None):
    import concourse.mybir as mybir
    from concourse.tile import TileContext

    mac_scan = _register_mac_scan()

    f16 = mybir.dt.float16
    f32 = mybir.dt.float32
    sub = mybir.AluOpType.subtract

    import concourse.bacc as bacc
    nc = bacc.Bacc(None, target_bir_lowering=False)
    u3t = nc.dram_tensor("u3t", [KP, MP], f16, kind="ExternalInput")
    # w3p pre-swizzled host-side to [p, kt*C] so the load is contiguous;
    # its c-axis is reordered [even c's | odd c's] so the build PSUM rows
    # land even-c in rows 0..47, odd-c in 48..95.
    w3p = nc.dram_tensor("w3p", [128, KT * C], f16, kind="ExternalInput")
    # nfa[p, t, cp, b128]: p = 64*(c%2) + i'; i'=48 row is ones
    nfa = nc.dram_tensor("nfa", [128, NT * PAIRS * 128], f16,
                         kind="ExternalInput")
    # nfprod[b, c*NW]: per-column weight nf_y(w)*nf_X(w)
    nfprod = nc.dram_tensor("nfprod", [B, C * NW], f16,
                            kind="ExternalInput")
    u2aug = nc.dram_tensor("u2aug", [32, NW], f16, kind="ExternalInput")
    w21 = nc.dram_tensor("w21", [32, C], f16, kind="ExternalInput")
    outp = nc.dram_tensor("out", [B, C], f32, kind="ExternalOutput")

    with TileContext(nc) as tc:
        with (
            nc.allow_low_precision(reason="fp16 intermediates; rel-err "
                                   "budget 2e-2 vs ~1e-3 incurred"),
            tc.tile_pool(name="dram", bufs=1, space="DRAM") as dpool,
            tc.tile_pool(name="const", bufs=1) as cpool,
            tc.tile_pool(name="u3", bufs=6) as u3pool,
            tc.tile_pool(name="psum", bufs=2, space="PSUM") as psum,
            tc.tile_pool(name="sc", bufs=2) as scpool,
            tc.tile_pool(name="nfr", bufs=2) as nfrpool,
            tc.tile_pool(name="nfat", bufs=2) as nfapool,
            tc.tile_pool(name="stg", bufs=3) as stgpool,
            tc.tile_pool(name="fin", bufs=2) as finpool,
        ):
            # scratch row c = [i-major build cols 0..7055 | aug 7056..7202]
            # rows: even c's at 0..47, odd at 48..95 (w3p reorder)
            w3u_scr = dpool.tile([C, SCR], f16)

            # ---- u3 stream + build own the sync queue; residents go on
            # the scalar queue; scratch writes on the gpsimd queue ----
            w3sb = cpool.tile([128, KT * C], f16)
            nc.sync.dma_start(out=w3sb[:, :], in_=w3p[:, :])
            w3v = w3sb[:, :].rearrange("p (k c) -> p k c", c=C)
            w21sb = cpool.tile([32, C], f16)
            nc.sync.dma_start(out=w21sb[:, :], in_=w21[:, :])
            u2sb = cpool.tile([32, NW], f16)
            nc.sync.dma_start(out=u2sb[:, :], in_=u2aug[:, :])

            nfav = nfa[:, :].rearrange("p (t m) -> p t m", t=NT)

            def load_nfa(t):
                nt = nfapool.tile([128, PAIRS * 128], f16, tag="nfa")
                for par in (0, 1):
                    r0 = 64 * par
                    nc.scalar.dma_start(out=nt[r0:r0 + I1, :],
                                        in_=nfav[r0:r0 + I1, t, :])
                return nt

            def load_nfprod(t):
                nt = nfrpool.tile([128, C * NW], f16, tag="nfr")
                nc.scalar.dma_start(out=nt[:, :],
                                    in_=nfprod[t * 128:(t + 1) * 128, :])
                return nt

            nfa_t = load_nfa(0)
            nfp_t = load_nfprod(0)

            # ---- aug build: [96, 147] = w21.T @ u2aug -> aug row ----
            aps = psum.tile([128, MCHUNK], f32, tag="z")
            nc.tensor.matmul(aps[:C, :NW], w21sb[:K2, :], u2sb[:K2, :],
                             start=True, stop=True)
            astg = stgpool.tile([C, MCHUNK], f16, tag="stg")
            nc.scalar.copy(astg[:, :NW], aps[:C, :NW])
            nc.gpsimd.dma_start(out=w3u_scr[:, MP:SCR], in_=astg[:, :NW])

            # ---- build: A2[c, m] = w3.T @ u3s, k-accumulated ----
            NMC = (MP + MCHUNK - 1) // MCHUNK  # 4 (2048,2048,2048,912)
            for mcp in range(NMC):
                wc = min(MCHUNK, MP - mcp * MCHUNK)
                ps = psum.tile([128, MCHUNK], f32, tag="z",
                               name=f"bp{mcp}")
                for kt in range(KT):
                    tl = u3pool.tile([128, MCHUNK], f16, tag="u3")
                    base = mcp * MCHUNK
                    nc.sync.dma_start(
                        out=tl[:, :wc],
                        in_=u3t[kt * 128:(kt + 1) * 128, base:base + wc])
                    for off in range(0, wc, 512):
                        h = min(512, wc - off)
                        nc.tensor.matmul(
                            ps[:C, off:off + h], w3v[:, kt, :],
                            tl[:, off:off + h],
                            start=(kt == 0), stop=(kt == KT - 1))
                stg = stgpool.tile([C, MCHUNK], f16, tag="stg")
                nc.scalar.copy(stg[:, :wc], ps[:C, :wc])
                nc.gpsimd.dma_start(
                    out=w3u_scr[:, mcp * MCHUNK:mcp * MCHUNK + wc],
                    in_=stg[:, :wc])

            # ---- lt: one resident tile, 2 big DMAs (even-c c0..c47 to
            # rows 0..48, odd-c to rows 64..112) ----
            lt = cpool.tile([128, PAIRS * NW], f16, name="ltbig")
            ltv = lt[:, :].rearrange("p (c m) -> p c m", m=NW)
            for par in (0, 1):
                rows = w3u_scr[48 * par:48 * par + 48, :].rearrange(
                    "c (i m) -> i c m", m=NW)
                nc.sync.dma_start(out=ltv[64 * par:64 * par + I1, :, :],
                                  in_=rows)

            # ---- phase B, t-major; unit = 4 consecutive c's ----
            nq = NQ if debug is None else debug
            for t in range(NT):
                if t + 1 < NT:
                    nfa_next = load_nfa(t + 1)
                    nfp_next = load_nfprod(t + 1)
                sct = scpool.tile([128, NQ * 4 * NW], f16, tag="sc")
                nfp_v = nfp_t[:, :].rearrange("p (c m) -> p c m", m=NW)
                for q in range(nq):
                    zt = psum.tile([128, MCHUNK], f32, tag="z",
                                   name=f"zt{t}_{q}")
                    for j in range(4):
                        cp = 2 * q + j // 2
                        ci = j % 2
                        lhsT = nfa_t[64 * ci:64 * ci + I1,
                                     cp * 128:(cp + 1) * 128]
                        rhs = ltv[64 * ci:64 * ci + I1, cp, :]
                        nc.tensor.matmul(zt[:, 512 * j:512 * j + NW],
                                         lhsT, rhs, start=True, stop=True)
                    zv = zt[:, :].rearrange(
                        "p (c n) -> p c n", n=512)[:, :, 0:NW]
                    rv = nfp_v[:, 4 * q:4 * q + 4, :]
                    ov = sct[:, q * 4 * NW:(q + 1) * 4 * NW].rearrange(
                        "p (c n) -> p c n", n=NW)
                    nc.vector._custom_dve(mac_scan, out=ov, in0=zv, in1=rv)

                # final: per 4c block: out_c0 = E0, out_cj = Ej - E(j-1)
                scv = sct[:, :].rearrange("p (q c m) -> p q c m",
                                          c=4, m=NW)
                ends = scv[:, :, :, NW - 1]
                ostf = finpool.tile([128, C], f32, tag="ostf")
                ostv = ostf[:, :].rearrange("p (q c) -> p q c", c=4)
                nc.vector.tensor_copy(ostv[:, :, 0:1], ends[:, :, 0:1])
                nc.vector.tensor_tensor(ostv[:, :, 1:4], ends[:, :, 1:4],
                                        ends[:, :, 0:3], sub)
                # undo the host-side 1/16 nfprod scaling (fp16 headroom)
                nc.vector.tensor_scalar_mul(ostf[:, :], ostf[:, :], 16.0)
                nc.gpsimd.dma_start(out=outp[t * 128:(t + 1) * 128, :],
                                    in_=ostf[:, :])
                if t + 1 < NT:
                    nfa_t = nfa_next
                    nfp_t = nfp_next
    nc.finalize()
    return nc


def _prep_inputs(node_feats, w3, w2, w1, U3, U2, U1):
    """Host-side fold, re-layout, fp16 casts, per-core sharding."""
    f16 = np.float16
    f32 = np.float32
    node_feats = np.asarray(node_feats, dtype=f32)

    # c-reorder for the build PSUM rows: [even c's | odd c's]
    c_perm = np.concatenate([np.arange(0, C, 2), np.arange(1, C, 2)])
    w3p = np.zeros((KP, C), dtype=f16)
    w3p[:K3] = np.asarray(w3, dtype=f32).astype(f16)[:, c_perm]
    # pre-swizzle to [p, kt*C] so the device load is contiguous
    w3p = np.ascontiguousarray(
        w3p.reshape(KT, 128, C).transpose(1, 0, 2).reshape(128, KT * C))
    w21 = np.zeros((32, C), dtype=f16)
    w21[:K2] = np.asarray(w2, dtype=f32).astype(f16)[:, c_perm]

    # nfa[p, t, cp, b]: p = 64*(c%2) + i'; i'=48 row is ones
    nf16 = node_feats.astype(f16)
    nfT = nf16.transpose(1, 2, 0)  # [c, i, b]
    nfa = np.zeros((128, NT, PAIRS, 128), dtype=f16)
    for par in (0, 1):
        s = nfT[par::2].transpose(1, 0, 2).reshape(I, PAIRS, NT, 128)
        nfa[64 * par:64 * par + I] = s.transpose(0, 2, 1, 3)
        nfa[64 * par + I] = 1.0
    nfa = np.ascontiguousarray(nfa.reshape(128, NT * PAIRS * 128))

    # fold U3: SYM[k, i, X, y] = U3w3-src folded over (x,y); diag halved
    U3_32 = np.asarray(U3, dtype=f32)
    u3_kixy = np.ascontiguousarray(U3_32.transpose(3, 2, 0, 1))
    SYM = (u3_kixy + u3_kixy.transpose(0, 1, 3, 2)).astype(f16)
    del u3_kixy
    U2f = np.asarray(U2, dtype=f32).transpose(2, 0, 1)
    U2S = (U2f + U2f.transpose(0, 2, 1)).astype(f16)

    # host U1 term (tiny): out1[b, c] = sum_x U1w1[c,x] nf[b,c,x]
    U1w1 = np.einsum("xk,kc->cx", np.asarray(U1, f32), np.asarray(w1, f32))
    host_out = np.einsum("cx,bcx->bc", U1w1,
                         node_feats.astype(np.float64))

    in_maps = []
    for r in range(NCORES):
        pairs = _core_pairs(r)
        u3a = np.zeros((KP, I, NW), dtype=f16)
        u2a = np.zeros((32, NW), dtype=f16)
        yidx = np.zeros(NW, dtype=np.int64)
        xidx = np.zeros(NW, dtype=np.int64)
        for g, (xa, xb) in enumerate(pairs):
            u3a[:K3, :, g * W:g * W + xa + 1] = SYM[:, :, xa, 0:xa + 1]
            u3a[:K3, :, g * W + xa] = SYM[:, :, xa, xa] / 2
            u2a[:K2, g * W:g * W + xa + 1] = U2S[:, xa, 0:xa + 1]
            u2a[:K2, g * W + xa] = U2S[:, xa, xa] / 2
            yidx[g * W:g * W + xa + 1] = np.arange(xa + 1)
            xidx[g * W:g * W + xa + 1] = xa
            u3a[:K3, :, g * W + xa + 1:g * W + W] = SYM[:, :, xb, 0:xb + 1]
            u3a[:K3, :, g * W + 48] = SYM[:, :, xb, xb] / 2
            u2a[:K2, g * W + xa + 1:g * W + W] = U2S[:, xb, 0:xb + 1]
            u2a[:K2, g * W + 48] = U2S[:, xb, xb] / 2
            yidx[g * W + xa + 1:g * W + W] = np.arange(xb + 1)
            xidx[g * W + xa + 1:g * W + W] = xb
        u3t = np.ascontiguousarray(u3a.reshape(KP, MP))

        # nfprod[b, c, w] = nf_y(w) * nf_X(w) / 16 (fp16 scan headroom;
        # the device rescales the final output by 16)
        nfprod = np.ascontiguousarray(
            (node_feats[:, :, yidx] * node_feats[:, :, xidx] * (1 / 16))
            .astype(f16).reshape(B, C * NW))

        in_maps.append({
            "u3t": u3t,
            "w3p": w3p,
            "nfa": nfa,
            "nfprod": nfprod,
            "u2aug": u2a,
            "w21": w21,
        })
    return in_maps, host_out


def kernel(node_feats, w3, w2, w1, U3, U2, U1):
    global LAST_EXEC_NS
    import os
    from concourse.bass_utils import run_bass_kernel_spmd

    if "nc" not in _CACHE:
        _CACHE["nc"] = _build_nc()
    nc = _CACHE["nc"]

    in_maps, host_out = _prep_inputs(node_feats, w3, w2, w1, U3, U2, U1)
    trace = bool(os.environ.get("BASS_TRACE"))
    res = run_bass_kernel_spmd(nc, in_maps, list(range(NCORES)),
                               trace=trace)
    LAST_EXEC_NS = res.exec_time_ns
    _CACHE["last_results"] = res

    out = host_out.copy()
    for r in range(NCORES):
        out += res.results[r]["out"].astype(np.float64)
    return out.astype(np.float32)
